# revision 1
# baseline (speedup 1.0000x reference)
"""GNN message-passing kernel for Trainium2 (8 NeuronCores, SPMD) — v3.

out = where(in_deg > 0, segment_sum(hidden[src], dst), hidden)
N=100000 nodes, E=1600000 edges, D=32 (hardcoded).

Design: edges sharded by dst range (core k owns rows [k*12500,(k+1)*12500)).
The per-edge *scatter* of v1 is eliminated: edges are grouped by destination
block of 256 rows, messages are fetched with dma_gather (the node table is
stored as bf16 hi/lo pairs so 128x bf16 rows = 256B, the minimum gather
payload), and the segment-sum is computed on the TensorEngine as
    psum[66, 256] += msg_chunk[128tok, 66].T @ onehot[128tok, 256]
where onehot[t, n] = (dstoff[t] == n) is built on the VectorEngine with a
broadcast is_equal.  Padding tokens carry dstoff = -1 which matches no
column, so they contribute exactly zero.  hi/lo bf16 splitting keeps f32-level
precision: products vs a {0,1} one-hot are exact in the f32 PSUM accumulate.

A final pass combines hi+lo, transposes each 128-row block back to row-major
on the TensorEngine, applies the isolated-node fixup (deg==0 -> keep hidden),
and writes the output shard.
"""

import os
import sys

import numpy as np

for _p in ("/opt/trn_rl_repo", os.path.expanduser("~/.axon_site/_ro/trn_rl_repo")):
    if os.path.isdir(_p) and _p not in sys.path:
        sys.path.insert(0, _p)

import ml_dtypes
import concourse.bacc as bacc
import concourse.mybir as mybir
from concourse import tile
from concourse.bass_utils import run_bass_kernel_spmd

N_NODES = 100000
N_EDGES = 1600000
D = 32
TROW = 128                    # bf16 table row: 32 hi | 1.0 | 32 lo | 0... (256B)
MCOL = 66                     # used message columns (32 hi + deg + 32 lo + lo-deg)

N_CORES = 8
DST_PER_CORE = N_NODES // N_CORES          # 12500
N_CHUNKS = 4
CHUNK = N_NODES // N_CHUNKS                # 25000 (int16 gather index limit)

BLK = 256                     # dst rows per psum block
NBLK = 49                     # ceil(12500/256)
CAP = 1280                    # tokens per (block, chunk) cell: mean ~1020 + 8 sigma
SLOTS = CAP // 128            # 10
NGRP = N_CHUNKS * NBLK        # 196 gather/matmul groups
OUT_ROWS = 12544              # 98*128 rows of output (12500 real + 44 junk)
NT = OUT_ROWS // 128          # 98 half-blocks

_cached = {}


def _build_program(loop_r=None, timing_mode=False):
    nc = bacc.Bacc(None, target_bir_lowering=False, debug=False,
                   num_swdge_queues=4, dynamic_dma_scratch_size=32768)
    f32 = mybir.dt.float32
    bf16 = mybir.dt.bfloat16
    i16 = mybir.dt.int16

    tab_d = nc.dram_tensor("tab", [N_NODES, TROW], bf16,
                           kind="Internal" if timing_mode else "ExternalInput")
    hid_d = nc.dram_tensor("hid", [OUT_ROWS, D], f32, kind="ExternalInput")
    sidx_d = nc.dram_tensor("sidx", [NGRP, 128, CAP // 16], i16,
                            kind="ExternalInput")
    doff_d = nc.dram_tensor("doff", [NGRP, 128, SLOTS], bf16,
                            kind="ExternalInput")
    iota_d = nc.dram_tensor("iota", [128, BLK], bf16, kind="ExternalInput")
    idn_d = nc.dram_tensor("idn", [MCOL, MCOL], f32, kind="ExternalInput")
    y_d = nc.dram_tensor("y", [OUT_ROWS, D], f32, kind="ExternalOutput")

    with tile.TileContext(nc) as tc:
        with (
            tc.tile_pool(name="cst", bufs=1) as cst_pool,
            tc.tile_pool(name="msg", bufs=6) as msg_pool,
            tc.tile_pool(name="idx", bufs=8) as idx_pool,
            tc.tile_pool(name="oh", bufs=4) as oh_pool,
            tc.tile_pool(name="acc", bufs=1) as acc_pool,
            tc.tile_pool(name="ps", bufs=4, space="PSUM") as ps_pool,
            tc.tile_pool(name="ps2", bufs=2, space="PSUM") as ps2_pool,
            tc.tile_pool(name="fix", bufs=1) as fix_pool,
        ):
            iota_t = cst_pool.tile([128, BLK], bf16)
            idn_t = cst_pool.tile([MCOL, MCOL], f32)
            nc.sync.dma_start(iota_t[:], iota_d[:])
            nc.sync.dma_start(idn_t[:], idn_d[:])
            acc_t = acc_pool.tile([MCOL, NBLK * BLK], f32)
            nc.vector.memset(acc_t[:], 0.0)

            def batch_phase(_i=None):
                for g in range(NGRP):
                    c, b = divmod(g, NBLK)
                    sidx_t = idx_pool.tile([128, CAP // 16], i16, tag="sidx")
                    doff_t = idx_pool.tile([128, SLOTS], bf16, tag="doff")
                    nc.sync.dma_start(sidx_t[:], sidx_d[g])
                    nc.sync.dma_start(doff_t[:], doff_d[g])
                    msg_t = msg_pool.tile([128, SLOTS, TROW], bf16, tag="msg")
                    nc.gpsimd.dma_gather(
                        msg_t[:], tab_d[c * CHUNK:(c + 1) * CHUNK, :],
                        sidx_t[:], CAP, CAP, TROW,
                        single_packet=False, queue_num=g % 4)
                    oh_t = oh_pool.tile([128, SLOTS, BLK], bf16, tag="oh")
                    nc.vector.tensor_tensor(
                        oh_t[:],
                        doff_t[:].unsqueeze(2).broadcast_to([128, SLOTS, BLK]),
                        iota_t[:].unsqueeze(1).broadcast_to([128, SLOTS, BLK]),
                        mybir.AluOpType.is_equal)
                    ps_t = ps_pool.tile([MCOL, BLK], f32, tag="ps")
                    for k in range(SLOTS):
                        nc.tensor.matmul(ps_t[:], msg_t[:, k, 0:MCOL],
                                         oh_t[:, k, :],
                                         start=(k == 0), stop=(k == SLOTS - 1))
                    sl = acc_t[:, b * BLK:(b + 1) * BLK]
                    nc.vector.tensor_add(sl, sl, ps_t[:])

            if loop_r is None:
                batch_phase()
            else:
                with tc.For_i(0, loop_r, 1) as _i:
                    batch_phase(_i)

            # phase B: combine hi+lo, transpose back to row-major, fixup
            hid_t = fix_pool.tile([128, NT, D], f32)
            y_t = fix_pool.tile([128, NT, D], f32)
            m_t = fix_pool.tile([128, NT], f32)
            nc.sync.dma_start(
                hid_t[:], hid_d.ap().rearrange("(t p) e -> p t e", p=128))
            for t in range(NT):
                ps2_t = ps2_pool.tile([128, MCOL], f32, tag="tr")
                nc.tensor.transpose(
                    ps2_t[:], acc_t[:, t * 128:(t + 1) * 128], idn_t[:])
                sb2_t = fix_pool.tile([128, MCOL], f32, tag="sb2", bufs=3)
                nc.vector.tensor_copy(sb2_t[:], ps2_t[:])
                # y = hi + lo
                nc.vector.tensor_add(y_t[:, t, :], sb2_t[:, 0:32],
                                     sb2_t[:, 33:65])
                # m = (deg == 0); y += m * hidden
                nc.vector.tensor_scalar(m_t[:, t:t + 1], sb2_t[:, 32:33],
                                        0.0, None, mybir.AluOpType.is_equal)
                nc.vector.tensor_scalar(
                    hid_t[:, t, :], hid_t[:, t, :], m_t[:, t:t + 1], None,
                    mybir.AluOpType.mult)
                nc.vector.tensor_add(y_t[:, t, :], y_t[:, t, :],
                                     hid_t[:, t, :])
            nc.sync.dma_start(
                y_d.ap().rearrange("(t p) e -> p t e", p=128), y_t[:])

    nc.compile()
    return nc


def _prep_inputs(hidden, src, dst):
    src = np.ascontiguousarray(src.astype(np.int64))
    dst = np.ascontiguousarray(dst.astype(np.int64))

    hi = hidden.astype(ml_dtypes.bfloat16)
    lo = (hidden - hi.astype(np.float32)).astype(ml_dtypes.bfloat16)
    tab = np.zeros((N_NODES, TROW), ml_dtypes.bfloat16)
    tab[:, 0:32] = hi
    tab[:, 32] = 1.0
    tab[:, 33:65] = lo

    owner = dst // DST_PER_CORE
    dst_local = dst - owner * DST_PER_CORE
    block = dst_local // BLK
    doffv = (dst_local - block * BLK).astype(np.float32)
    chunk = src // CHUNK

    cell = (owner * N_CHUNKS + chunk) * NBLK + block    # [E]
    order = np.argsort(cell, kind="stable")
    sc = cell[order]
    cs = np.r_[0, np.nonzero(np.diff(sc))[0] + 1]
    csizes = np.diff(np.r_[cs, len(sc)])
    if csizes.max() > CAP:
        raise RuntimeError(f"cell overflow: {csizes.max()} > CAP={CAP}")
    rank_sorted = np.arange(len(sc)) - np.repeat(cs, csizes)
    rank = np.empty(len(sc), np.int64)
    rank[order] = rank_sorted

    src16 = np.zeros((N_CORES, NGRP, CAP), np.int16)
    doff = np.full((N_CORES, NGRP, CAP), -1.0, ml_dtypes.bfloat16)
    grp = chunk * NBLK + block
    src16[owner, grp, rank] = (src - chunk * CHUNK).astype(np.int16)
    doff[owner, grp, rank] = doffv.astype(ml_dtypes.bfloat16)

    # gather idx layout: token t -> [t % 16, t // 16], replicated x8
    w = src16.reshape(N_CORES, NGRP, CAP // 16, 16)
    w = np.ascontiguousarray(np.moveaxis(w, -1, -2))
    src16w = np.tile(w, (1, 1, 8, 1))
    # dstoff layout: token t -> [t % 128, t // 128] (msg tile layout)
    doffw = np.ascontiguousarray(
        np.moveaxis(doff.reshape(N_CORES, NGRP, SLOTS, 128), -1, -2))

    iota = np.tile(np.arange(BLK, dtype=np.float32).astype(
        ml_dtypes.bfloat16)[None, :], (128, 1))
    idn = np.eye(MCOL, dtype=np.float32)

    in_maps = []
    for k in range(N_CORES):
        hid = np.zeros((OUT_ROWS, D), np.float32)
        hid[:DST_PER_CORE] = hidden[k * DST_PER_CORE:(k + 1) * DST_PER_CORE]
        in_maps.append({
            "tab": tab,
            "hid": hid,
            "sidx": np.ascontiguousarray(src16w[k]),
            "doff": np.ascontiguousarray(doffw[k]),
            "iota": iota,
            "idn": idn,
        })
    return in_maps


def kernel(hidden, src, dst, **run_kwargs):
    if "nc" not in _cached:
        _cached["nc"] = _build_program()
    nc = _cached["nc"]
    in_maps = _prep_inputs(np.asarray(hidden, np.float32), src, dst)
    res = run_bass_kernel_spmd(nc, in_maps, core_ids=list(range(N_CORES)),
                               **run_kwargs)
    out = np.concatenate(
        [res.results[k]["y"][:DST_PER_CORE] for k in range(N_CORES)], axis=0)
    if run_kwargs:
        _cached["last_results"] = res
    return out

